# revision 17
# baseline (speedup 1.0000x reference)
"""Trainium2 Bass kernel for nn_CrossAttention_block (B=16, Tq=Tk=1024, d_model=24, 4 heads x 6).

Strategy (data-parallel over batch, 2 batches per core on 8 cores):
  - The mask I_m has no effect in the reference (torch masked_fill bug faithfully
    reproduced), so it is never shipped to the device.
  - Transposed layout throughout: host supplies X^T / Xen^T per batch; the device
    computes Q^T/K^T spread so all 4 heads sit at 32-partition offsets of a single
    [128, T] tile, so QK^T score matmuls (contraction dim = 6) run row-tiled on
    the PE array and AV matmuls run col-tiled, with softmax denominators coming
    free from a ones-column appended to V.
  - Softmax: scores ~ N(0,1) so exp never overflows -> no max subtraction.
    exp runs on ACT over [128,1024] PSUM tiles with the 1/sqrt(6) scale folded in.
  - Software pipelining: AV matmuls for granule g are emitted after exp of
    granule g+1 so the PE never sits behind an ACT dependency in its FIFO;
    the epilogue is likewise emitted lagged into the next (b,qc) block.
  - Epilogue without transposes: denominators are read strided from the O PSUM
    tile, reciprocal'd on DVE, broadcast across partitions by a tiny PE matmul
    with a 0/1 expansion matrix, multiplied elementwise on DVE, then projected
    with a spread Wo (zero rows kill the inter-head garbage lanes).
"""

import math
import sys

import numpy as np

if "/opt/trn_rl_repo" not in sys.path:
    sys.path.insert(0, "/opt/trn_rl_repo")

N_CORES = 8
B, T, D = 16, 1024, 24
H, HD = 4, 6
BPC = B // N_CORES  # batches per core
SCALE = 1.0 / math.sqrt(HD)

# matmul dtype for the attention matmuls: "f32" or "bf16"
DT_MODE = "bf16"
# number of times the attention body is emitted (timing experiments only)
REPEAT = 1
# which pieces of the attention body to emit (timing ablations only)
PARTS = "qk,exp,av,epi"

_CACHE = {}


def _build(dt_mode, repeat=1, parts="qk,exp,av,epi"):
    parts = set(parts.split(","))
    import concourse.tile as tile
    from concourse import bacc, mybir

    f32 = mybir.dt.float32
    bf16 = mybir.dt.bfloat16
    mdt = bf16 if dt_mode == "bf16" else f32

    nc = bacc.Bacc(None)
    XT = nc.declare_dram_parameter("XT", [BPC, D, T], f32, isOutput=False)
    XenT = nc.declare_dram_parameter("XenT", [BPC, D, T], f32, isOutput=False)
    WqSp = nc.declare_dram_parameter("WqSp", [D, 128], f32, isOutput=False)
    WkSp = nc.declare_dram_parameter("WkSp", [D, 128], f32, isOutput=False)
    WvT = nc.declare_dram_parameter("WvT", [D, D], f32, isOutput=False)
    WoSp = nc.declare_dram_parameter("WoSp", [128, D], mdt, isOutput=False)
    E4 = nc.declare_dram_parameter("E4", [128, 128], mdt, isOutput=False)
    YT = nc.declare_dram_parameter("YT", [BPC, D, T], f32, isOutput=True)

    with tile.TileContext(nc) as tc:
        from contextlib import ExitStack

        with ExitStack() as es:
            es.enter_context(
                nc.allow_low_precision(
                    reason="softmax weights in bf16; 2e-2 rel tolerance"
                )
            )
            cp = es.enter_context(tc.tile_pool(name="const", bufs=1))
            pp = es.enter_context(tc.tile_pool(name="ptil", bufs=3))
            op = es.enter_context(tc.tile_pool(name="epi", bufs=2))
            psS = es.enter_context(tc.tile_pool(name="psS", bufs=2, space="PSUM"))
            psO = es.enter_context(tc.tile_pool(name="psO", bufs=2, space="PSUM"))
            psE = es.enter_context(tc.tile_pool(name="psE", bufs=1, space="PSUM"))

            wqsp = cp.tile([D, 128], f32, tag="wqsp")
            nc.sync.dma_start(wqsp[:], WqSp[:])
            wksp = cp.tile([D, 128], f32, tag="wksp")
            nc.sync.dma_start(wksp[:], WkSp[:])
            wvt = cp.tile([D, D], f32, tag="wvt")
            nc.sync.dma_start(wvt[:], WvT[:])
            wosp = cp.tile([128, D], mdt, tag="wosp")
            nc.sync.dma_start(wosp[:], WoSp[:])
            e4 = cp.tile([128, 128], mdt, tag="e4")
            nc.sync.dma_start(e4[:], E4[:])
            # persistent reciprocal staging tile: row 32h carries head h's
            # denominator reciprocals; other rows stay at the memset value so
            # the expansion matmul never reads uninitialized SBUF
            rec128 = cp.tile([128, 512], mdt, tag="rec128")
            nc.vector.memset(rec128[:], 1.0)

            # ---- projections (outside the timed body) ----
            qts, kts, vaugs = [], [], []
            for b in range(BPC):
                xt = cp.tile([D, T], f32, tag=f"xt{b}")
                nc.sync.dma_start(xt[:], XT[b])
                xent = cp.tile([D, T], f32, tag=f"xent{b}")
                nc.sync.dma_start(xent[:], XenT[b])

                qt = cp.tile([128, T], mdt, tag=f"qt{b}")
                kt = cp.tile([128, T], mdt, tag=f"kt{b}")
                for qc in range(2):
                    qs = slice(512 * qc, 512 * (qc + 1))
                    for wsp, src, dst in ((wqsp, xt, qt), (wksp, xent, kt)):
                        prj = psE.tile([128, 512], f32, tag="rx", name="prj")
                        nc.tensor.matmul(
                            prj[:], lhsT=wsp[:], rhs=src[:, qs], start=True, stop=True
                        )
                        nc.vector.tensor_copy(dst[:, qs], prj[:])
                qts.append(qt)
                kts.append(kt)

                # V with ones column: [128, 8 chunks x (4 heads x 7)]
                vaug = cp.tile([128, 8 * 28], mdt, tag=f"vaug{b}")
                nc.vector.memset(vaug[:], 1.0)
                for t in range(8):
                    vps = psE.tile([128, D], f32, tag="rx", name="vps")
                    nc.tensor.matmul(
                        vps[:],
                        lhsT=xent[:, 128 * t : 128 * (t + 1)],
                        rhs=wvt[:],
                        start=True,
                        stop=True,
                    )
                    # head layout [ones, v0..v5]: denominator lands on the
                    # 32-aligned partition 32h of o_ps (PSUM base alignment)
                    nc.vector.tensor_copy(
                        vaug[:, 28 * t : 28 * (t + 1)].rearrange(
                            "p (h x) -> p h x", h=4
                        )[:, :, 1:7],
                        vps.rearrange("p (h x) -> p h x", h=4),
                    )
                vaugs.append(vaug)

            # ---- attention main loop (the timed body) ----
            prept = []
            if "av" in parts and "exp" not in parts:
                for i in range(3):
                    t_ = pp.tile([128, 1024], mdt, tag="p")
                    nc.vector.memset(t_[:], 0.001)
                    prept.append(t_)
            ptctr = [0]

            pend_av = [None]  # (pt, b, pair, t, o_ps)
            pend_epi = [None]  # (b, qc, o_ps)
            o_init = [0]  # first-touch memsets so inter-head garbage lanes are finite

            def flush_av():
                if pend_av[0] is None:
                    return
                pt, b_, pair_, t_, ops_ = pend_av[0]
                pend_av[0] = None
                if "av" not in parts:
                    return
                for i in range(2):
                    h = 2 * pair_ + i
                    nc.tensor.matmul(
                        ops_[32 * h : 32 * h + 7, :],
                        lhsT=vaugs[b_][:, 28 * t_ + 7 * h : 28 * t_ + 7 * h + 7],
                        rhs=pt[:, 512 * i : 512 * (i + 1)],
                        start=(t_ == 0),
                        stop=(t_ == 7),
                        tile_position=(0, 32 * h),
                    )

            def flush_epi():
                if pend_epi[0] is None:
                    return
                b_, qc_, ops_ = pend_epi[0]
                pend_epi[0] = None
                if "epi" not in parts:
                    return
                qs_ = slice(512 * qc_, 512 * (qc_ + 1))
                for h in range(H):
                    nc.vector.reciprocal(
                        rec128[32 * h : 32 * h + 1, :], ops_[32 * h : 32 * h + 1, :]
                    )
                rx = psE.tile([128, 512], f32, tag="rx", name="rx")
                nc.tensor.matmul(
                    rx[:], lhsT=e4[:], rhs=rec128[:], start=True, stop=True
                )
                rx_sb = op.tile([128, 512], mdt, tag="rxsb")
                nc.vector.tensor_copy(rx_sb[:], rx[:])
                onrm = op.tile([128, 512], mdt, tag="on")
                nc.vector.tensor_mul(onrm[:], ops_[:], rx_sb[:])
                y_ps = psE.tile([D, 512], f32, tag="y", name="y")
                nc.tensor.matmul(
                    y_ps[:], lhsT=wosp[:], rhs=onrm[:], start=True, stop=True
                )
                y_sb = op.tile([D, 512], f32, tag="ysb")
                nc.vector.tensor_copy(y_sb[:], y_ps[:])
                nc.sync.dma_start(YT[b_][:, qs_], y_sb[:])

            for _rep in range(repeat):
                for b in range(BPC):
                    qt, kt = qts[b], kts[b]
                    for qc in range(2):
                        qs = slice(512 * qc, 512 * (qc + 1))
                        o_ps = psO.tile([128, 512], f32, tag="o")
                        if o_init[0] < 2:
                            o_init[0] += 1
                            nc.vector.memset(o_ps[:], 0.0)
                        for pair in range(2):
                            for t in range(8):
                                if "qk" in parts:
                                    stile = psS.tile([128, 1024], f32, tag="s")
                                    for i in range(2):
                                        h = 2 * pair + i
                                        nc.tensor.matmul(
                                            stile[:, 512 * i : 512 * (i + 1)],
                                            lhsT=kt[
                                                32 * h : 32 * h + HD,
                                                128 * t : 128 * (t + 1),
                                            ],
                                            rhs=qt[32 * h : 32 * h + HD, qs],
                                            start=True,
                                            stop=True,
                                            tile_position=(32 * h, 0),
                                        )
                                if "exp" in parts:
                                    pt = pp.tile([128, 1024], mdt, tag="p")
                                    nc.scalar.activation(
                                        pt[:],
                                        stile[:],
                                        mybir.ActivationFunctionType.Exp,
                                        scale=SCALE,
                                    )
                                else:
                                    pt = prept[ptctr[0] % 3] if prept else None
                                    ptctr[0] += 1
                                flush_av()
                                if pt is not None:
                                    pend_av[0] = (pt, b, pair, t, o_ps)
                                if pair == 0 and t == 1:
                                    flush_epi()
                        pend_epi[0] = (b, qc, o_ps)
            flush_av()
            flush_epi()

    nc.compile()
    return nc


def _make_runner(nc, n_cores=N_CORES):
    """Build the sharded PJRT callable once; reuse across kernel() calls so
    repeat calls skip retracing and NEFF reload."""
    import jax
    from jax.experimental.shard_map import shard_map
    from jax.sharding import Mesh, NamedSharding, PartitionSpec

    from concourse import bass2jax, mybir

    bass2jax.install_neuronx_cc_hook()
    partition_name = nc.partition_id_tensor.name if nc.partition_id_tensor else None

    in_names, out_names, out_avals, zero_outs = [], [], [], []
    for alloc in nc.m.functions[0].allocations:
        if not isinstance(alloc, mybir.MemoryLocationSet):
            continue
        name = alloc.memorylocations[0].name
        if alloc.kind == "ExternalInput":
            if name != partition_name:
                in_names.append(name)
        elif alloc.kind == "ExternalOutput":
            out_names.append(name)
            shape = tuple(alloc.tensor_shape)
            dtype = mybir.dt.np(alloc.dtype)
            out_avals.append(jax.core.ShapedArray(shape, dtype))
            zero_outs.append(np.zeros(shape, dtype))
    n_params = len(in_names)
    n_outs = len(out_avals)
    all_in_names = list(in_names) + list(out_names)
    if partition_name is not None:
        all_in_names.append(partition_name)

    def _body(*args):
        operands = list(args)
        if partition_name is not None:
            operands.append(bass2jax.partition_id_tensor())
        return tuple(
            bass2jax._bass_exec_p.bind(
                *operands,
                out_avals=tuple(out_avals),
                in_names=tuple(all_in_names),
                out_names=tuple(out_names),
                lowering_input_output_aliases=(),
                sim_require_finite=True,
                sim_require_nnan=True,
                nc=nc,
            )
        )

    devices = jax.devices()[:n_cores]
    mesh = Mesh(np.asarray(devices), ("core",))
    in_specs = (PartitionSpec("core"),) * (n_params + n_outs)
    out_specs = (PartitionSpec("core"),) * len(out_names)
    fn = jax.jit(
        shard_map(_body, mesh=mesh, in_specs=in_specs, out_specs=out_specs,
                  check_rep=False),
        keep_unused=True,
    )
    sharding = NamedSharding(mesh, PartitionSpec("core"))
    concat_zeros = [
        jax.device_put(
            np.zeros((n_cores * z.shape[0], *z.shape[1:]), z.dtype), sharding
        )
        for z in zero_outs
    ]

    def run(in_maps):
        staged = [
            jax.device_put(
                np.concatenate(
                    [np.asarray(in_maps[c][nm]) for c in range(n_cores)], axis=0
                ),
                sharding,
            )
            for nm in in_names
        ]
        out_arrs = [np.asarray(a) for a in fn(*staged, *concat_zeros)]
        return [
            {
                name: out_arrs[i].reshape(n_cores, *out_avals[i].shape)[c]
                for i, name in enumerate(out_names)
            }
            for c in range(n_cores)
        ]

    return run


def _get_runner():
    key = (DT_MODE, REPEAT, PARTS)
    if key not in _CACHE:
        _CACHE[key] = _make_runner(_build(DT_MODE, REPEAT, PARTS))
    return _CACHE[key]


def _np_mdt():
    if DT_MODE == "bf16":
        import ml_dtypes

        return ml_dtypes.bfloat16
    return np.float32


def kernel(X, X_en, I_m=None, Wq=None, Wk=None, Wv=None, Wo=None):
    X = np.ascontiguousarray(np.asarray(X, np.float32))
    X_en = np.ascontiguousarray(np.asarray(X_en, np.float32))
    Wq = np.asarray(Wq, np.float32)
    Wk = np.asarray(Wk, np.float32)
    Wv = np.asarray(Wv, np.float32)
    Wo = np.asarray(Wo, np.float32)
    mdt = _np_mdt()

    XT_all = np.ascontiguousarray(X.transpose(0, 2, 1))
    XenT_all = np.ascontiguousarray(X_en.transpose(0, 2, 1))

    # WqSp[d, 32h+c] = Wq[6h+c, d]  (c < 6; other columns zero)
    wqsp = np.zeros((D, 128), np.float32)
    wksp = np.zeros((D, 128), np.float32)
    for h in range(H):
        wqsp[:, 32 * h : 32 * h + HD] = Wq[HD * h : HD * (h + 1), :].T
        wksp[:, 32 * h : 32 * h + HD] = Wk[HD * h : HD * (h + 1), :].T
    # WoSp[32h+1+c, i] = Wo[i, 6h+c]  (c < 6; row 32h is the denominator lane;
    # other rows zero)
    wosp = np.zeros((128, D), np.float32)
    for h in range(H):
        wosp[32 * h + 1 : 32 * h + 1 + HD, :] = Wo[:, HD * h : HD * (h + 1)].T
    # E4[32h, 32h:32h+32] = 1 (expansion: rx row r = rec128 row 32*(r//32))
    e4 = np.zeros((128, 128), np.float32)
    for h in range(H):
        e4[32 * h, 32 * h : 32 * (h + 1)] = 1.0

    shared = {
        "WqSp": wqsp,
        "WkSp": wksp,
        "WvT": np.ascontiguousarray(Wv.T),
        "WoSp": wosp.astype(mdt),
        "E4": e4.astype(mdt),
    }
    in_maps = [
        {
            "XT": XT_all[BPC * c : BPC * (c + 1)],
            "XenT": XenT_all[BPC * c : BPC * (c + 1)],
            **shared,
        }
        for c in range(N_CORES)
    ]
    res = _get_runner()(in_maps)
    Y = np.concatenate([r["YT"].transpose(0, 2, 1) for r in res], axis=0)
    return np.ascontiguousarray(Y, dtype=np.float32)


# revision 21
# speedup vs baseline: 2.0523x; 2.0523x over previous
"""Trainium2 Bass kernel for nn_CrossAttention_block (B=16, Tq=Tk=1024, d_model=24, 4 heads x 6).

Strategy (data-parallel over batch, 2 batches per core on 8 cores):
  - The mask I_m has no effect in the reference (torch masked_fill bug faithfully
    reproduced), so it is never shipped to the device.
  - Transposed layout throughout: host supplies X^T / Xen^T per batch; the device
    computes Q^T/K^T spread so all 4 heads sit at 32-partition offsets of a single
    [128, T] tile, so QK^T score matmuls (contraction dim = 6) run row-tiled on
    the PE array and AV matmuls run col-tiled, with softmax denominators coming
    free from a ones-column appended to V.
  - Softmax: scores ~ N(0,1) so exp never overflows -> no max subtraction.
    exp runs on ACT over [128,1024] PSUM tiles with the 1/sqrt(6) scale folded in.
  - Software pipelining: AV matmuls for granule g are emitted after exp of
    granule g+1 so the PE never sits behind an ACT dependency in its FIFO;
    the epilogue is likewise emitted lagged into the next (b,qc) block.
  - Epilogue without transposes: denominators are read strided from the O PSUM
    tile, reciprocal'd on DVE, broadcast across partitions by a tiny PE matmul
    with a 0/1 expansion matrix, multiplied elementwise on DVE, then projected
    with a spread Wo (zero rows kill the inter-head garbage lanes).
"""

import math
import sys

import numpy as np

if "/opt/trn_rl_repo" not in sys.path:
    sys.path.insert(0, "/opt/trn_rl_repo")

N_CORES = 8
B, T, D = 16, 1024, 24
H, HD = 4, 6
BPC = B // N_CORES  # batches per core
SCALE = 1.0 / math.sqrt(HD)

# matmul dtype for the attention matmuls: "f32" or "bf16"
DT_MODE = "bf16"
# number of times the attention body is emitted (timing experiments only)
REPEAT = 1
# which pieces of the attention body to emit (timing ablations only)
PARTS = "qk,exp,av,epi"

_CACHE = {}


def _build(dt_mode, repeat=1, parts="qk,exp,av,epi"):
    parts = set(parts.split(","))
    import concourse.tile as tile
    from concourse import bacc, mybir

    f32 = mybir.dt.float32
    bf16 = mybir.dt.bfloat16
    mdt = bf16 if dt_mode == "bf16" else f32

    nc = bacc.Bacc(None)
    XT = nc.declare_dram_parameter("XT", [BPC, D, T], f32, isOutput=False)
    XenT = nc.declare_dram_parameter("XenT", [BPC, D, T], f32, isOutput=False)
    WqSp = nc.declare_dram_parameter("WqSp", [D, 128], f32, isOutput=False)
    WkSp = nc.declare_dram_parameter("WkSp", [D, 128], f32, isOutput=False)
    WvT = nc.declare_dram_parameter("WvT", [D, D], f32, isOutput=False)
    WoSp = nc.declare_dram_parameter("WoSp", [128, D], mdt, isOutput=False)
    E4 = nc.declare_dram_parameter("E4", [128, 128], mdt, isOutput=False)
    YT = nc.declare_dram_parameter("YT", [BPC, D, T], f32, isOutput=True)

    with tile.TileContext(nc) as tc:
        from contextlib import ExitStack

        with ExitStack() as es:
            es.enter_context(
                nc.allow_low_precision(
                    reason="softmax weights in bf16; 2e-2 rel tolerance"
                )
            )
            cp = es.enter_context(tc.tile_pool(name="const", bufs=1))
            pp = es.enter_context(tc.tile_pool(name="ptil", bufs=4))
            op = es.enter_context(tc.tile_pool(name="epi", bufs=2))
            psS = es.enter_context(tc.tile_pool(name="psS", bufs=2, space="PSUM"))
            psO = es.enter_context(tc.tile_pool(name="psO", bufs=2, space="PSUM"))
            psE = es.enter_context(tc.tile_pool(name="psE", bufs=1, space="PSUM"))

            wqsp = cp.tile([D, 128], f32, tag="wqsp")
            nc.sync.dma_start(wqsp[:], WqSp[:])
            wksp = cp.tile([D, 128], f32, tag="wksp")
            nc.sync.dma_start(wksp[:], WkSp[:])
            wvt = cp.tile([D, D], f32, tag="wvt")
            nc.sync.dma_start(wvt[:], WvT[:])
            wosp = cp.tile([128, D], mdt, tag="wosp")
            nc.sync.dma_start(wosp[:], WoSp[:])
            e4 = cp.tile([128, 128], mdt, tag="e4")
            nc.sync.dma_start(e4[:], E4[:])
            # persistent reciprocal staging tile: row 32h carries head h's
            # denominator reciprocals; other rows stay at the memset value so
            # the expansion matmul never reads uninitialized SBUF
            rec128 = cp.tile([128, 512], mdt, tag="rec128")
            nc.vector.memset(rec128[:], 1.0)

            # ---- projections (outside the timed body) ----
            qts, kts, vaugs = [], [], []
            for b in range(BPC):
                xt = cp.tile([D, T], f32, tag=f"xt{b}")
                nc.sync.dma_start(xt[:], XT[b])
                xent = cp.tile([D, T], f32, tag=f"xent{b}")
                nc.sync.dma_start(xent[:], XenT[b])

                qt = cp.tile([128, T], mdt, tag=f"qt{b}")
                kt = cp.tile([128, T], mdt, tag=f"kt{b}")
                for qc in range(2):
                    qs = slice(512 * qc, 512 * (qc + 1))
                    for wsp, src, dst in ((wqsp, xt, qt), (wksp, xent, kt)):
                        prj = psE.tile([128, 512], f32, tag="rx", name="prj")
                        nc.tensor.matmul(
                            prj[:], lhsT=wsp[:], rhs=src[:, qs], start=True, stop=True
                        )
                        nc.vector.tensor_copy(dst[:, qs], prj[:])
                qts.append(qt)
                kts.append(kt)

                # V with ones column: [128, 8 chunks x (4 heads x 7)]
                vaug = cp.tile([128, 8 * 28], mdt, tag=f"vaug{b}")
                nc.vector.memset(vaug[:], 1.0)
                for t in range(8):
                    vps = psE.tile([128, D], f32, tag="rx", name="vps")
                    nc.tensor.matmul(
                        vps[:],
                        lhsT=xent[:, 128 * t : 128 * (t + 1)],
                        rhs=wvt[:],
                        start=True,
                        stop=True,
                    )
                    # head layout [ones, v0..v5]: denominator lands on the
                    # 32-aligned partition 32h of o_ps (PSUM base alignment)
                    nc.vector.tensor_copy(
                        vaug[:, 28 * t : 28 * (t + 1)].rearrange(
                            "p (h x) -> p h x", h=4
                        )[:, :, 1:7],
                        vps.rearrange("p (h x) -> p h x", h=4),
                    )
                vaugs.append(vaug)

            # ---- attention main loop (the timed body) ----
            prept = []
            if "av" in parts and "exp" not in parts:
                for i in range(3):
                    t_ = pp.tile([128, 1024], mdt, tag="p")
                    nc.vector.memset(t_[:], 0.001)
                    prept.append(t_)
            ptctr = [0]

            AV_LAG = 2  # granules between exp(g) and its AV matmuls
            pend_av = []  # deque of (pt, b, pair, t, o_ps)
            pend_epi = [None]  # (b, qc, o_ps)
            o_init = [0]  # first-touch memsets so inter-head garbage lanes are finite

            def flush_av(limit):
                while len(pend_av) > limit:
                    pt, b_, pair_, t_, ops_ = pend_av.pop(0)
                    if "av" not in parts:
                        continue
                    for i in range(2):
                        h = 2 * pair_ + i
                        nc.tensor.matmul(
                            ops_[32 * h : 32 * h + 7, :],
                            lhsT=vaugs[b_][:, 28 * t_ + 7 * h : 28 * t_ + 7 * h + 7],
                            rhs=pt[:, 512 * i : 512 * (i + 1)],
                            start=(t_ == 0),
                            stop=(t_ == 7),
                            tile_position=(0, 32 * h),
                        )

            def flush_epi():
                if pend_epi[0] is None:
                    return
                b_, qc_, ops_ = pend_epi[0]
                pend_epi[0] = None
                if "epi" not in parts:
                    return
                qs_ = slice(512 * qc_, 512 * (qc_ + 1))
                for h in range(H):
                    nc.vector.reciprocal(
                        rec128[32 * h : 32 * h + 1, :], ops_[32 * h : 32 * h + 1, :]
                    )
                rx = psE.tile([128, 512], f32, tag="rx", name="rx")
                nc.tensor.matmul(
                    rx[:], lhsT=e4[:], rhs=rec128[:], start=True, stop=True
                )
                rx_sb = op.tile([128, 512], mdt, tag="rxsb")
                nc.vector.tensor_copy(rx_sb[:], rx[:])
                onrm = op.tile([128, 512], mdt, tag="on")
                nc.vector.tensor_mul(onrm[:], ops_[:], rx_sb[:])
                y_ps = psE.tile([D, 512], f32, tag="y", name="y")
                nc.tensor.matmul(
                    y_ps[:], lhsT=wosp[:], rhs=onrm[:], start=True, stop=True
                )
                y_sb = op.tile([D, 512], f32, tag="ysb")
                nc.vector.tensor_copy(y_sb[:], y_ps[:])
                nc.sync.dma_start(YT[b_][:, qs_], y_sb[:])

            for _rep in range(repeat):
                for b in range(BPC):
                    qt, kt = qts[b], kts[b]
                    for qc in range(2):
                        qs = slice(512 * qc, 512 * (qc + 1))
                        o_ps = psO.tile([128, 512], f32, tag="o")
                        if o_init[0] < 2:
                            o_init[0] += 1
                            nc.vector.memset(o_ps[:], 0.0)
                        for pair in range(2):
                            for t in range(8):
                                if "qk" in parts:
                                    stile = psS.tile([128, 1024], f32, tag="s")
                                    for i in range(2):
                                        h = 2 * pair + i
                                        nc.tensor.matmul(
                                            stile[:, 512 * i : 512 * (i + 1)],
                                            lhsT=kt[
                                                32 * h : 32 * h + HD,
                                                128 * t : 128 * (t + 1),
                                            ],
                                            rhs=qt[32 * h : 32 * h + HD, qs],
                                            start=True,
                                            stop=True,
                                            tile_position=(32 * h, 0),
                                        )
                                if "exp" in parts:
                                    pt = pp.tile([128, 1024], mdt, tag="p")
                                    nc.scalar.activation(
                                        pt[:],
                                        stile[:],
                                        mybir.ActivationFunctionType.Exp,
                                        scale=SCALE,
                                    )
                                else:
                                    pt = prept[ptctr[0] % 3] if prept else None
                                    ptctr[0] += 1
                                if pt is not None:
                                    pend_av.append((pt, b, pair, t, o_ps))
                                flush_av(AV_LAG)
                                if pair == 0 and t == 4:
                                    flush_epi()
                        pend_epi[0] = (b, qc, o_ps)
            flush_av(0)
            flush_epi()

    nc.compile()
    return nc


def _make_runner(nc, n_cores=N_CORES):
    """Build the sharded PJRT callable once; reuse across kernel() calls so
    repeat calls skip retracing and NEFF reload."""
    import jax
    from jax.experimental.shard_map import shard_map
    from jax.sharding import Mesh, NamedSharding, PartitionSpec

    from concourse import bass2jax, mybir

    bass2jax.install_neuronx_cc_hook()
    partition_name = nc.partition_id_tensor.name if nc.partition_id_tensor else None

    in_names, out_names, out_avals, zero_outs = [], [], [], []
    for alloc in nc.m.functions[0].allocations:
        if not isinstance(alloc, mybir.MemoryLocationSet):
            continue
        name = alloc.memorylocations[0].name
        if alloc.kind == "ExternalInput":
            if name != partition_name:
                in_names.append(name)
        elif alloc.kind == "ExternalOutput":
            out_names.append(name)
            shape = tuple(alloc.tensor_shape)
            dtype = mybir.dt.np(alloc.dtype)
            out_avals.append(jax.core.ShapedArray(shape, dtype))
            zero_outs.append(np.zeros(shape, dtype))
    n_params = len(in_names)
    n_outs = len(out_avals)
    all_in_names = list(in_names) + list(out_names)
    if partition_name is not None:
        all_in_names.append(partition_name)

    def _body(*args):
        operands = list(args)
        if partition_name is not None:
            operands.append(bass2jax.partition_id_tensor())
        return tuple(
            bass2jax._bass_exec_p.bind(
                *operands,
                out_avals=tuple(out_avals),
                in_names=tuple(all_in_names),
                out_names=tuple(out_names),
                lowering_input_output_aliases=(),
                sim_require_finite=True,
                sim_require_nnan=True,
                nc=nc,
            )
        )

    devices = jax.devices()[:n_cores]
    mesh = Mesh(np.asarray(devices), ("core",))
    in_specs = (PartitionSpec("core"),) * (n_params + n_outs)
    out_specs = (PartitionSpec("core"),) * len(out_names)
    fn = jax.jit(
        shard_map(_body, mesh=mesh, in_specs=in_specs, out_specs=out_specs,
                  check_rep=False),
        keep_unused=True,
    )
    sharding = NamedSharding(mesh, PartitionSpec("core"))
    concat_zeros = [
        jax.device_put(
            np.zeros((n_cores * z.shape[0], *z.shape[1:]), z.dtype), sharding
        )
        for z in zero_outs
    ]

    def run(in_maps):
        staged = [
            jax.device_put(
                np.concatenate(
                    [np.asarray(in_maps[c][nm]) for c in range(n_cores)], axis=0
                ),
                sharding,
            )
            for nm in in_names
        ]
        out_arrs = [np.asarray(a) for a in fn(*staged, *concat_zeros)]
        return [
            {
                name: out_arrs[i].reshape(n_cores, *out_avals[i].shape)[c]
                for i, name in enumerate(out_names)
            }
            for c in range(n_cores)
        ]

    return run


def _get_runner():
    key = (DT_MODE, REPEAT, PARTS)
    if key not in _CACHE:
        _CACHE[key] = _make_runner(_build(DT_MODE, REPEAT, PARTS))
    return _CACHE[key]


def _np_mdt():
    if DT_MODE == "bf16":
        import ml_dtypes

        return ml_dtypes.bfloat16
    return np.float32


def kernel(X, X_en, I_m=None, Wq=None, Wk=None, Wv=None, Wo=None):
    X = np.ascontiguousarray(np.asarray(X, np.float32))
    X_en = np.ascontiguousarray(np.asarray(X_en, np.float32))
    Wq = np.asarray(Wq, np.float32)
    Wk = np.asarray(Wk, np.float32)
    Wv = np.asarray(Wv, np.float32)
    Wo = np.asarray(Wo, np.float32)
    mdt = _np_mdt()

    XT_all = np.ascontiguousarray(X.transpose(0, 2, 1))
    XenT_all = np.ascontiguousarray(X_en.transpose(0, 2, 1))

    # WqSp[d, 32h+c] = Wq[6h+c, d]  (c < 6; other columns zero)
    wqsp = np.zeros((D, 128), np.float32)
    wksp = np.zeros((D, 128), np.float32)
    for h in range(H):
        wqsp[:, 32 * h : 32 * h + HD] = Wq[HD * h : HD * (h + 1), :].T
        wksp[:, 32 * h : 32 * h + HD] = Wk[HD * h : HD * (h + 1), :].T
    # WoSp[32h+1+c, i] = Wo[i, 6h+c]  (c < 6; row 32h is the denominator lane;
    # other rows zero)
    wosp = np.zeros((128, D), np.float32)
    for h in range(H):
        wosp[32 * h + 1 : 32 * h + 1 + HD, :] = Wo[:, HD * h : HD * (h + 1)].T
    # E4[32h, 32h:32h+32] = 1 (expansion: rx row r = rec128 row 32*(r//32))
    e4 = np.zeros((128, 128), np.float32)
    for h in range(H):
        e4[32 * h, 32 * h : 32 * (h + 1)] = 1.0

    shared = {
        "WqSp": wqsp,
        "WkSp": wksp,
        "WvT": np.ascontiguousarray(Wv.T),
        "WoSp": wosp.astype(mdt),
        "E4": e4.astype(mdt),
    }
    in_maps = [
        {
            "XT": XT_all[BPC * c : BPC * (c + 1)],
            "XenT": XenT_all[BPC * c : BPC * (c + 1)],
            **shared,
        }
        for c in range(N_CORES)
    ]
    res = _get_runner()(in_maps)
    Y = np.concatenate([r["YT"].transpose(0, 2, 1) for r in res], axis=0)
    return np.ascontiguousarray(Y, dtype=np.float32)
